# revision 21
# baseline (speedup 1.0000x reference)
"""Trainium2 Bass kernel for nn_Attention_41566693491235.

Computes, for full inputs (B=256, L=196, R=1024, A=512, D=2048):
    att_h  = h @ W_h + b_h                                  [B, A]
    dot    = einsum("bla,a->bl", tanh(f2 + att_h[:,None,:]), w_a) + b_a
    weight = softmax(dot, axis=1) * mask;  weight /= weight.sum(1, keepdims=True)
    att    = einsum("bl,bld->bd", weight, f1)               [B, D]

Sharding: data-parallel over batch, 32 per core x 8 cores. Weights replicated.
Note b_a and the softmax normalizer cancel exactly in the masked renorm:
    weight = exp(dot - max) * mask / sum(exp(dot - max) * mask)

Implementation notes:
  - Matmuls run in float32r (full-rate PE at moving-dim >= 256, ~2e-4 rel err).
  - The final weighted sum packs 4 batches per 128-row contraction via
    block-diagonal stationary operands (rows k = j*4 + bs), so all 32 output
    rows accumulate partition-aligned in one PSUM tile.
  - The kernel is software-pipelined over quads of 4 batches: tanh+dot+softmax
    and the weight-redistribution round-trip for quad q overlap the f1
    streaming matmuls of quad q-1. Small latency-chain DMAs ride on gpsimd
    (SWDGE) so they never head-of-line-block the bulk HWDGE streams.
"""

import numpy as np

import concourse.bass as bass
import concourse.bacc as bacc
import concourse.tile as tile
import concourse.mybir as mybir
from concourse import bass_utils

F32 = mybir.dt.float32
F32R = mybir.dt.float32r
AF = mybir.ActivationFunctionType

# Problem shape (hardcoded; kernel.py must be self-contained).
B, L, R, A, D = 256, 196, 1024, 512, 2048
NCORES = 8
BL = B // NCORES          # 32 batches per core
NQ = BL // 4              # 8 quads of 4 batches
LFULL = (L // 32) * 32    # 192: l-range covered by full 32-row chunks
NLC = LFULL // 32         # 6 full l-chunks per quad
LRAG = L - LFULL          # 4: ragged l rows
KRAG = 4 * LRAG           # 16: ragged contraction rows (4 l x 4 batches)
NAC = A // 128            # 4 chunks of the attention-hidden dim
NKC = R // 128            # 8 chunks of the h-feature dim
NDC = D // 512            # 4 free-dim chunks for the output matmuls


def _build_program(f1_bufs: int = 13):
    nc = bacc.Bacc(
        "TRN2",
        target_bir_lowering=False,
        debug=False,
        enable_asserts=False,
        num_devices=NCORES,
    )

    hT = nc.dram_tensor("hT", [R, BL], F32, kind="ExternalInput").ap()
    wh = nc.dram_tensor("wh", [R, A], F32, kind="ExternalInput").ap()
    bh4 = nc.dram_tensor("bh4", [128, NAC], F32, kind="ExternalInput").ap()
    wa4 = nc.dram_tensor("wa4", [128, NAC], F32, kind="ExternalInput").ap()
    # f2h[bp, p, i, ac, l] = f2[2*bp+i, l, ac*128+p]: one contiguous
    # 800KB block per batch-pair, 6.3KB per partition.
    f2h = nc.dram_tensor("f2h", [BL // 2, 128, 2, NAC, L], F32,
                         kind="ExternalInput").ap()
    # f1h[q, lc, j, bs, d] = f1[4q+bs, 32lc+j, d]: each (q, lc) tile is a
    # contiguous 1MB block in exactly the SBUF layout.
    f1h = nc.dram_tensor("f1h", [NQ, NLC, 128, D], F32,
                         kind="ExternalInput").ap()
    f1hr = nc.dram_tensor("f1hr", [NQ, KRAG, D], F32,
                          kind="ExternalInput").ap()
    # msk[bs, q, l] = att_masks[4q+bs, l]: per-quad slices at partitions 0-3.
    msk = nc.dram_tensor("msk", [4, NQ, L], F32, kind="ExternalInput").ap()
    bdm = nc.dram_tensor("bdm", [NQ, 128, BL], F32, kind="ExternalInput").ap()
    bdmr = nc.dram_tensor("bdmr", [NQ, KRAG, BL], F32, kind="ExternalInput").ap()
    att = nc.dram_tensor("att", [BL, D], F32, kind="ExternalOutput").ap()

    with tile.TileContext(nc) as tc:
        with (
            tc.tile_pool(name="const", bufs=1) as cpool,
            tc.tile_pool(name="f2p", bufs=3) as f2pool,
            tc.tile_pool(name="ep", bufs=3) as epool,
            tc.tile_pool(name="f1p", bufs=f1_bufs) as f1pool,
            tc.tile_pool(name="f1rp", bufs=2) as f1rpool,
            tc.tile_pool(name="small", bufs=1) as spool,
            tc.tile_pool(name="ps", bufs=1, space=bass.MemorySpace.PSUM) as pspool,
            tc.tile_pool(name="psdot", bufs=2, space=bass.MemorySpace.PSUM) as psdot,
            tc.tile_pool(name="dram", bufs=1, space=bass.MemorySpace.DRAM) as dpool,
        ):
            # ---- constants -------------------------------------------------
            wh_t = cpool.tile([128, NKC, A], F32R)
            nc.sync.dma_start(wh_t[:], wh.bitcast(F32R).rearrange("(kc p) a -> p kc a", p=128))
            hT_t = cpool.tile([128, NKC, BL], F32R)
            nc.sync.dma_start(hT_t[:], hT.bitcast(F32R).rearrange("(kc p) b -> p kc b", p=128))
            wa_t = cpool.tile([128, NAC], F32R)
            nc.sync.dma_start(wa_t[:], wa4.bitcast(F32R)[:])
            bh_t = cpool.tile([128, NAC], F32)
            nc.scalar.dma_start(bh_t[:], bh4[:])
            msk_t = cpool.tile([4, NQ, L], F32)
            nc.scalar.dma_start(msk_t[:], msk[:])
            bdm_t = cpool.tile([128, NQ, BL], F32)
            nc.scalar.dma_start(bdm_t[:], bdm.rearrange("q k b -> k q b"))
            bdmr_t = cpool.tile([KRAG, NQ, BL], F32)
            nc.scalar.dma_start(bdmr_t[:], bdmr.rearrange("q k b -> k q b"))

            # DRAM scratch for the partition-redistribution round-trips.
            dot_dram = dpool.tile([NQ, 4 * L], F32)
            w_dram = dpool.tile([NQ, NLC, 32, 4], F32)
            w_dram_r = dpool.tile([NQ, LRAG, 4], F32)

            # ---- phase 1: att_h.T = W_h.T @ h.T (+ b_h) --------------------
            # atth[:, ac, b] holds att_h[b, ac*128 + p] on partition p.
            ps_atth = pspool.tile([128, NAC, BL], F32)
            for mc in range(NAC):
                for kc in range(NKC):
                    nc.tensor.matmul(
                        ps_atth[:, mc, :],
                        wh_t[:, kc, mc * 128:(mc + 1) * 128],
                        hT_t[:, kc, :],
                        start=(kc == 0),
                        stop=(kc == NKC - 1),
                    )
            atth = cpool.tile([128, NAC, BL], F32)
            for mc in range(NAC):
                nc.vector.tensor_scalar_add(
                    atth[:, mc, :], ps_atth[:, mc, :], bh_t[:, mc:mc + 1]
                )

            # Output accumulator: all 32 rows, 4 x 512 free chunks (4 banks).
            ps_att = pspool.tile([BL, NDC, 512], F32)

            f1_tiles = {}

            def emit_front(q):
                """tanh + dot + softmax + weight redistribution for quad q,
                plus the f1 prefetch for quad q."""
                dotflat = spool.tile([1, 4 * L], F32, tag="dotflat", bufs=2,
                                     name=f"dotflat{q}")
                for i2 in range(2):
                    bp = 2 * q + i2
                    f2b = f2pool.tile([128, 2, NAC, L], F32, tag="f2b",
                                      name=f"f2b{bp}")
                    dma_eng = nc.sync if bp % 2 == 0 else nc.scalar
                    dma_eng.dma_start(f2b[:], f2h[bp])
                    e2 = epool.tile([128, NAC, 2, L], F32R, tag="e2",
                                    name=f"e2{bp}")
                    for i in range(2):
                        for ac in range(NAC):
                            nc.scalar.activation(
                                e2[:, ac, i, :], f2b[:, i, ac, :], AF.Tanh,
                                bias=atth[:, ac, 2 * bp + i:2 * bp + i + 1],
                            )
                    pd = psdot.tile([1, 2, L], F32, tag="pd", name=f"pd{bp}")
                    for ac in range(NAC):
                        nc.tensor.matmul(
                            pd[:], wa_t[:, ac:ac + 1], e2[:, ac, :, :],
                            start=(ac == 0), stop=(ac == NAC - 1),
                        )
                    nc.vector.tensor_copy(
                        dotflat[0:1, i2 * 2 * L:(i2 + 1) * 2 * L], pd[:]
                    )

                # f1 prefetch for this quad (bulk HWDGE streams).
                for lc in range(NLC):
                    f1t = f1pool.tile([128, D], F32R, tag="f1t",
                                      name=f"f1t{q}_{lc}")
                    dma_eng = nc.sync if (q * NLC + lc) % 2 == 0 else nc.scalar
                    dma_eng.dma_start(f1t[:], f1h.bitcast(F32R)[q, lc])
                    f1_tiles[(q, lc)] = f1t
                f1r = f1rpool.tile([KRAG, D], F32R, tag="f1r", name=f"f1r{q}")
                nc.sync.dma_start(f1r[:], f1hr.bitcast(F32R)[q])
                f1_tiles[(q, "r")] = f1r

                # Small latency-chain DMAs ride on gpsimd (SWDGE).
                nc.gpsimd.dma_start(dot_dram[q], dotflat[:])
                dott = spool.tile([4, L], F32, tag="dott", bufs=2,
                                  name=f"dott{q}")
                nc.gpsimd.dma_start(dott[:], dot_dram[q].rearrange("(b l) -> b l", l=L))

                negmax = spool.tile([4, 1], F32, tag="negmax", bufs=2,
                                    name=f"negmax{q}")
                nc.vector.tensor_reduce(
                    negmax[:], dott[:], axis=mybir.AxisListType.X,
                    op=mybir.AluOpType.max, negate=True,
                )
                wexp = spool.tile([4, L], F32, tag="wexp", bufs=2,
                                  name=f"wexp{q}")
                nc.scalar.activation(wexp[:], dott[:], AF.Exp, bias=negmax[:])
                wm = spool.tile([4, L], F32, tag="wm", bufs=2, name=f"wm{q}")
                nc.vector.tensor_mul(wm[:], wexp[:], msk_t[:, q, :])
                ssum = spool.tile([4, 1], F32, tag="ssum", bufs=2,
                                  name=f"ssum{q}")
                nc.vector.reduce_sum(ssum[:], wm[:], axis=mybir.AxisListType.X)
                sinv = spool.tile([4, 1], F32, tag="sinv", bufs=2,
                                  name=f"sinv{q}")
                nc.vector.reciprocal(sinv[:], ssum[:])
                wn = spool.tile([4, L], F32, tag="wn", bufs=2, name=f"wn{q}")
                nc.vector.tensor_scalar_mul(wn[:], wm[:], sinv[:])

                # Redistribute wn[bs, 32lc+j] -> W2[(j bs), lc] via DRAM.
                nc.gpsimd.dma_start(
                    w_dram[q].rearrange("lc j bs -> bs lc j"), wn[:, 0:LFULL]
                )
                nc.gpsimd.dma_start(
                    w_dram_r[q].rearrange("j bs -> bs j"), wn[:, LFULL:L]
                )
                w2 = spool.tile([128, NLC], F32, tag="w2", bufs=2,
                                name=f"w2{q}")
                nc.gpsimd.dma_start(
                    w2[:], w_dram[q].rearrange("lc j bs -> (j bs) lc")
                )
                w2r = spool.tile([KRAG, 1], F32, tag="w2r", bufs=2,
                                 name=f"w2r{q}")
                nc.gpsimd.dma_start(
                    w2r[:], w_dram_r[q].rearrange("j bs -> (j bs)")
                )
                # Block-diagonal stationary operands.
                ldt = spool.tile([128, NLC, BL], F32R, tag="ldt", bufs=2,
                                 name=f"ldt{q}")
                for lc in range(NLC):
                    nc.vector.tensor_scalar_mul(
                        ldt[:, lc, :], bdm_t[:, q, :], w2[:, lc:lc + 1]
                    )
                ldr = spool.tile([KRAG, BL], F32R, tag="ldr", bufs=2,
                                 name=f"ldr{q}")
                nc.vector.tensor_scalar_mul(ldr[:], bdmr_t[:, q, :], w2r[:])
                return ldt, ldr

            def emit_step5(q, ldt, ldr):
                for lc in range(NLC):
                    f1t = f1_tiles.pop((q, lc))
                    for dc in range(NDC):
                        nc.tensor.matmul(
                            ps_att[:, dc, :],
                            ldt[:, lc, :],
                            f1t[:, dc * 512:(dc + 1) * 512],
                            start=(q == 0 and lc == 0),
                            stop=False,
                        )
                f1r = f1_tiles.pop((q, "r"))
                for dc in range(NDC):
                    nc.tensor.matmul(
                        ps_att[:, dc, :],
                        ldr[:],
                        f1r[:, dc * 512:(dc + 1) * 512],
                        start=False,
                        stop=(q == NQ - 1),
                    )

            # Software pipeline with one-quad skew.
            prev = None
            for q in range(NQ):
                cur = emit_front(q)
                if prev is not None:
                    emit_step5(q - 1, *prev)
                prev = cur
            emit_step5(NQ - 1, *prev)

            for dc in range(NDC):
                att_sb = spool.tile([BL, 512], F32, tag="att_sb", bufs=2,
                                    name=f"att_sb{dc}")
                nc.vector.tensor_copy(att_sb[:], ps_att[:, dc, :])
                nc.sync.dma_start(att[:, dc * 512:(dc + 1) * 512], att_sb[:])

    nc.compile()
    return nc


_PROGRAM_CACHE = {}


def _get_program():
    if "nc" not in _PROGRAM_CACHE:
        _PROGRAM_CACHE["nc"] = _build_program()
    return _PROGRAM_CACHE["nc"]


def _block_diag_masks():
    bdm = np.zeros((NQ, 128, BL), dtype=np.float32)
    bdmr = np.zeros((NQ, KRAG, BL), dtype=np.float32)
    for q in range(NQ):
        for bs in range(4):
            bdm[q, bs::4, 4 * q + bs] = 1.0        # rows k = j*4 + bs
            bdmr[q, bs::4, 4 * q + bs] = 1.0
    return bdm, bdmr


def make_in_maps(h, att_feats1, att_feats2, att_masks, W_h, b_h, w_a, b_a):
    h = np.asarray(h, dtype=np.float32)
    att_feats1 = np.asarray(att_feats1, dtype=np.float32)
    att_feats2 = np.asarray(att_feats2, dtype=np.float32)
    att_masks = np.asarray(att_masks, dtype=np.float32)
    W_h = np.ascontiguousarray(np.asarray(W_h, dtype=np.float32))
    b_h = np.asarray(b_h, dtype=np.float32)
    w_a = np.asarray(w_a, dtype=np.float32)
    del b_a  # cancels exactly in the softmax + masked renormalization

    wa4 = np.ascontiguousarray(w_a.reshape(NAC, 128).T)
    bh4 = np.ascontiguousarray(b_h.reshape(NAC, 128).T)
    bdm, bdmr = _block_diag_masks()

    in_maps = []
    for c in range(NCORES):
        sl = slice(c * BL, (c + 1) * BL)
        f1c = att_feats1[sl]
        f2c = att_feats2[sl]
        # f2h[bp, p, i, ac, l] = f2[2bp+i, l, ac*128+p]
        f2h = np.ascontiguousarray(
            f2c.reshape(BL // 2, 2, L, NAC, 128).transpose(0, 4, 1, 3, 2)
        )
        # f1h[q, lc, j, bs, d] = f1[4q+bs, 32lc+j, d]
        f1h = np.ascontiguousarray(
            f1c[:, :LFULL, :].reshape(NQ, 4, NLC, 32, D).transpose(0, 2, 3, 1, 4)
        ).reshape(NQ, NLC, 128, D)
        f1hr = np.ascontiguousarray(
            f1c[:, LFULL:, :].reshape(NQ, 4, LRAG, D).transpose(0, 2, 1, 3)
        ).reshape(NQ, KRAG, D)
        in_maps.append({
            "hT": np.ascontiguousarray(h[sl].T),
            "wh": W_h,
            "bh4": bh4,
            "wa4": wa4,
            "f2h": f2h,
            "f1h": f1h,
            "f1hr": f1hr,
            "msk": np.ascontiguousarray(
                att_masks[sl].reshape(NQ, 4, L).transpose(1, 0, 2)
            ),
            "bdm": bdm,
            "bdmr": bdmr,
        })
    return in_maps


def kernel(h, att_feats1, att_feats2, att_masks, W_h, b_h, w_a, b_a,
           _trace=False, _return_results=False):
    nc = _get_program()
    in_maps = make_in_maps(h, att_feats1, att_feats2, att_masks, W_h, b_h,
                           w_a, b_a)
    res = bass_utils.run_bass_kernel_spmd(
        nc, in_maps, core_ids=list(range(NCORES)), trace=_trace
    )
    out = np.concatenate([res.results[c]["att"] for c in range(NCORES)], axis=0)
    if _return_results:
        return out, res
    return out


# revision 31
# speedup vs baseline: 1.1972x; 1.1972x over previous
"""Trainium2 Bass kernel for nn_Attention_41566693491235.

Computes, for full inputs (B=256, L=196, R=1024, A=512, D=2048):
    att_h  = h @ W_h + b_h                                  [B, A]
    dot    = einsum("bla,a->bl", tanh(f2 + att_h[:,None,:]), w_a) + b_a
    weight = softmax(dot, axis=1) * mask;  weight /= weight.sum(1, keepdims=True)
    att    = einsum("bl,bld->bd", weight, f1)               [B, D]

Sharding: data-parallel over batch, 32 per core x 8 cores. Weights replicated.
Note b_a and the softmax normalizer cancel exactly in the masked renorm:
    weight = exp(dot - max) * mask / sum(exp(dot - max) * mask)

Implementation notes:
  - Matmuls run in float32r (full-rate PE at moving-dim >= 256, ~2e-4 rel err).
  - The final weighted sum packs 4 batches per 128-row contraction via
    block-diagonal stationary operands (rows k = j*4 + bs), so all 32 output
    rows accumulate partition-aligned in one PSUM tile.
  - The kernel is software-pipelined over quads of 4 batches: tanh+dot+softmax
    and the weight-redistribution round-trip for quad q overlap the f1
    streaming matmuls of quad q-1. Small latency-chain DMAs ride on gpsimd
    (SWDGE) so they never head-of-line-block the bulk HWDGE streams.
"""

import numpy as np

import concourse.bass as bass
import concourse.bacc as bacc
import concourse.tile as tile
import concourse.mybir as mybir
from concourse import bass_utils

F32 = mybir.dt.float32
F32R = mybir.dt.float32r
AF = mybir.ActivationFunctionType

# Problem shape (hardcoded; kernel.py must be self-contained).
B, L, R, A, D = 256, 196, 1024, 512, 2048
NCORES = 8
BL = B // NCORES          # 32 batches per core
NQ = BL // 4              # 8 quads of 4 batches
LFULL = (L // 32) * 32    # 192: l-range covered by full 32-row chunks
NLC = LFULL // 32         # 6 full l-chunks per quad
LRAG = L - LFULL          # 4: ragged l rows
KRAG = 4 * LRAG           # 16: ragged contraction rows (4 l x 4 batches)
NAC = A // 128            # 4 chunks of the attention-hidden dim
NKC = R // 128            # 8 chunks of the h-feature dim
NDC = D // 512            # 4 free-dim chunks for the output matmuls


def _build_program(f1_bufs: int = 12):
    nc = bacc.Bacc(
        "TRN2",
        target_bir_lowering=False,
        debug=False,
        enable_asserts=False,
        num_devices=NCORES,
    )

    hT = nc.dram_tensor("hT", [R, BL], F32, kind="ExternalInput").ap()
    wh = nc.dram_tensor("wh", [R, A], F32, kind="ExternalInput").ap()
    bh4 = nc.dram_tensor("bh4", [128, NAC], F32, kind="ExternalInput").ap()
    wa4 = nc.dram_tensor("wa4", [128, NAC], F32, kind="ExternalInput").ap()
    # f2h[bp, p, i, ac, l] = f2[2*bp+i, l, ac*128+p]: one contiguous
    # 800KB block per batch-pair, 6.3KB per partition.
    f2h = nc.dram_tensor("f2h", [BL // 2, 128, 2, NAC, L], F32,
                         kind="ExternalInput").ap()
    # f1h[q, lc, j, bs, d] = f1[4q+bs, 32lc+j, d]: each (q, lc) tile is a
    # contiguous 1MB block in exactly the SBUF layout.
    f1h = nc.dram_tensor("f1h", [NQ, NLC, 128, D], F32,
                         kind="ExternalInput").ap()
    f1hr = nc.dram_tensor("f1hr", [NQ, KRAG, D], F32,
                          kind="ExternalInput").ap()
    # msk[bs, q, l] = att_masks[4q+bs, l]: per-quad slices at partitions 0-3.
    msk = nc.dram_tensor("msk", [4, NQ, L], F32, kind="ExternalInput").ap()
    bdm = nc.dram_tensor("bdm", [NQ, 128, BL], F32, kind="ExternalInput").ap()
    bdmr = nc.dram_tensor("bdmr", [NQ, KRAG, BL], F32, kind="ExternalInput").ap()
    att = nc.dram_tensor("att", [BL, D], F32, kind="ExternalOutput").ap()

    with tile.TileContext(nc) as tc:
        with (
            tc.tile_pool(name="const", bufs=1) as cpool,
            tc.tile_pool(name="f2p", bufs=3) as f2pool,
            tc.tile_pool(name="ep", bufs=3) as epool,
            tc.tile_pool(name="f1p", bufs=f1_bufs) as f1pool,
            tc.tile_pool(name="f1rp", bufs=1) as f1rpool,
            tc.tile_pool(name="small", bufs=1) as spool,
            tc.tile_pool(name="ps", bufs=1, space=bass.MemorySpace.PSUM) as pspool,
            tc.tile_pool(name="psdot", bufs=3, space=bass.MemorySpace.PSUM) as psdot,
            tc.tile_pool(name="dram", bufs=1, space=bass.MemorySpace.DRAM) as dpool,
        ):
            # ---- constants -------------------------------------------------
            wh_t = cpool.tile([128, NKC, A], F32R)
            nc.sync.dma_start(wh_t[:], wh.bitcast(F32R).rearrange("(kc p) a -> p kc a", p=128))
            hT_t = cpool.tile([128, NKC, BL], F32R)
            nc.sync.dma_start(hT_t[:], hT.bitcast(F32R).rearrange("(kc p) b -> p kc b", p=128))
            wa_t = cpool.tile([128, NAC], F32R)
            nc.sync.dma_start(wa_t[:], wa4.bitcast(F32R)[:])
            bh_t = cpool.tile([128, NAC], F32)
            nc.scalar.dma_start(bh_t[:], bh4[:])
            msk_t = cpool.tile([4, NQ, L], F32)
            nc.scalar.dma_start(msk_t[:], msk[:])
            bdm_t = cpool.tile([128, NQ, BL], F32)
            nc.scalar.dma_start(bdm_t[:], bdm.rearrange("q k b -> k q b"))
            bdmr_t = cpool.tile([KRAG, NQ, BL], F32)
            nc.scalar.dma_start(bdmr_t[:], bdmr.rearrange("q k b -> k q b"))

            # DRAM scratch for the partition-redistribution round-trips.
            dot_dram = dpool.tile([NQ, 4 * L], F32)
            w_dram = dpool.tile([NQ, NLC, 32, 4], F32)
            w_dram_r = dpool.tile([NQ, LRAG, 4], F32)

            # ---- phase 1: att_h.T = W_h.T @ h.T (+ b_h) --------------------
            # atth[:, ac, b] holds att_h[b, ac*128 + p] on partition p.
            ps_atth = pspool.tile([128, NAC, BL], F32)
            for mc in range(NAC):
                for kc in range(NKC):
                    nc.tensor.matmul(
                        ps_atth[:, mc, :],
                        wh_t[:, kc, mc * 128:(mc + 1) * 128],
                        hT_t[:, kc, :],
                        start=(kc == 0),
                        stop=(kc == NKC - 1),
                    )
            atth = cpool.tile([128, NAC, BL], F32)
            for mc in range(NAC):
                nc.vector.tensor_scalar_add(
                    atth[:, mc, :], ps_atth[:, mc, :], bh_t[:, mc:mc + 1]
                )

            # Output accumulator: all 32 rows, 4 x 512 free chunks (4 banks).
            ps_att = pspool.tile([BL, NDC, 512], F32)

            f1_tiles = {}

            def emit_front(q):
                """tanh + dot + softmax + weight redistribution for quad q,
                plus the f1 prefetch for quad q."""
                dotflat = spool.tile([1, 4 * L], F32, tag="dotflat", bufs=2,
                                     name=f"dotflat{q}")
                for i2 in range(2):
                    bp = 2 * q + i2
                    f2b = f2pool.tile([128, 2, NAC, L], F32, tag="f2b",
                                      name=f"f2b{bp}")
                    # f2 rides the scalar HWDGE ring; f1 owns the sync ring,
                    # so slot-waits on one never block the other.
                    nc.scalar.dma_start(f2b[:], f2h[bp])
                    e2 = epool.tile([128, NAC, 2, L], F32R, tag="e2",
                                    name=f"e2{bp}")
                    for i in range(2):
                        for ac in range(NAC):
                            nc.scalar.activation(
                                e2[:, ac, i, :], f2b[:, i, ac, :], AF.Tanh,
                                bias=atth[:, ac, 2 * bp + i:2 * bp + i + 1],
                            )
                    pd = psdot.tile([1, 2, L], F32, tag="pd", name=f"pd{bp}")
                    for ac in range(NAC):
                        nc.tensor.matmul(
                            pd[:], wa_t[:, ac:ac + 1], e2[:, ac, :, :],
                            start=(ac == 0), stop=(ac == NAC - 1),
                        )
                    nc.vector.tensor_copy(
                        dotflat[0:1, i2 * 2 * L:(i2 + 1) * 2 * L], pd[:]
                    )

                # f1 prefetch for this quad (bulk HWDGE streams).
                for lc in range(NLC):
                    f1t = f1pool.tile([128, D], F32R, tag="f1t",
                                      name=f"f1t{q}_{lc}")
                    nc.sync.dma_start(f1t[:], f1h.bitcast(F32R)[q, lc])
                    f1_tiles[(q, lc)] = f1t

                # Small latency-chain DMAs ride on gpsimd (SWDGE).
                nc.gpsimd.dma_start(dot_dram[q], dotflat[:])
                dott = spool.tile([4, L], F32, tag="dott", bufs=2,
                                  name=f"dott{q}")
                nc.gpsimd.dma_start(dott[:], dot_dram[q].rearrange("(b l) -> b l", l=L))

                negmax = spool.tile([4, 1], F32, tag="negmax", bufs=2,
                                    name=f"negmax{q}")
                nc.vector.tensor_reduce(
                    negmax[:], dott[:], axis=mybir.AxisListType.X,
                    op=mybir.AluOpType.max, negate=True,
                )
                wexp = spool.tile([4, L], F32, tag="wexp", bufs=2,
                                  name=f"wexp{q}")
                nc.scalar.activation(wexp[:], dott[:], AF.Exp, bias=negmax[:])
                wm = spool.tile([4, L], F32, tag="wm", bufs=2, name=f"wm{q}")
                nc.vector.tensor_mul(wm[:], wexp[:], msk_t[:, q, :])
                ssum = spool.tile([4, 1], F32, tag="ssum", bufs=2,
                                  name=f"ssum{q}")
                nc.vector.reduce_sum(ssum[:], wm[:], axis=mybir.AxisListType.X)
                sinv = spool.tile([4, 1], F32, tag="sinv", bufs=2,
                                  name=f"sinv{q}")
                nc.vector.reciprocal(sinv[:], ssum[:])
                wn = spool.tile([4, L], F32, tag="wn", bufs=2, name=f"wn{q}")
                nc.vector.tensor_scalar_mul(wn[:], wm[:], sinv[:])

                # Redistribute wn[bs, 32lc+j] -> W2[(j bs), lc] via DRAM.
                nc.gpsimd.dma_start(
                    w_dram[q].rearrange("lc j bs -> bs lc j"), wn[:, 0:LFULL]
                )
                nc.gpsimd.dma_start(
                    w_dram_r[q].rearrange("j bs -> bs j"), wn[:, LFULL:L]
                )
                w2 = spool.tile([128, NLC], F32, tag="w2", bufs=2,
                                name=f"w2{q}")
                nc.gpsimd.dma_start(
                    w2[:], w_dram[q].rearrange("lc j bs -> (j bs) lc")
                )
                w2r = spool.tile([KRAG, 1], F32, tag="w2r", bufs=2,
                                 name=f"w2r{q}")
                nc.gpsimd.dma_start(
                    w2r[:], w_dram_r[q].rearrange("j bs -> (j bs)")
                )
                # Block-diagonal stationary operands.
                ldt = spool.tile([128, NLC, BL], F32R, tag="ldt", bufs=3,
                                 name=f"ldt{q}")
                for lc in range(NLC):
                    nc.vector.tensor_scalar_mul(
                        ldt[:, lc, :], bdm_t[:, q, :], w2[:, lc:lc + 1]
                    )
                ldr = spool.tile([KRAG, BL], F32R, tag="ldr", bufs=3,
                                 name=f"ldr{q}")
                nc.vector.tensor_scalar_mul(ldr[:], bdmr_t[:, q, :], w2r[:])
                return ldt, ldr

            def emit_step5(q, ldt, ldr):
                # Ragged f1 rows load just-in-time: they're consumed at the
                # end of this quad's matmul burst.
                f1r = f1rpool.tile([KRAG, D], F32R, tag="f1r", name=f"f1r{q}")
                nc.sync.dma_start(f1r[:], f1hr.bitcast(F32R)[q])
                f1_tiles[(q, "r")] = f1r
                for lc in range(NLC):
                    f1t = f1_tiles.pop((q, lc))
                    for dc in range(NDC):
                        nc.tensor.matmul(
                            ps_att[:, dc, :],
                            ldt[:, lc, :],
                            f1t[:, dc * 512:(dc + 1) * 512],
                            start=(q == 0 and lc == 0),
                            stop=False,
                        )
                f1r = f1_tiles.pop((q, "r"))
                for dc in range(NDC):
                    nc.tensor.matmul(
                        ps_att[:, dc, :],
                        ldr[:],
                        f1r[:, dc * 512:(dc + 1) * 512],
                        start=False,
                        stop=(q == NQ - 1),
                    )

            # Software pipeline with two-quad skew: the cross-engine
            # softmax/redistribution chain for quad q has two quad-cycles to
            # complete before step5(q) needs its result.
            lds = {}
            for q in range(NQ):
                lds[q] = emit_front(q)
                if q >= 2:
                    emit_step5(q - 2, *lds.pop(q - 2))
            emit_step5(NQ - 2, *lds.pop(NQ - 2))
            emit_step5(NQ - 1, *lds.pop(NQ - 1))

            for dc in range(NDC):
                att_sb = spool.tile([BL, 512], F32, tag="att_sb", bufs=2,
                                    name=f"att_sb{dc}")
                nc.vector.tensor_copy(att_sb[:], ps_att[:, dc, :])
                nc.sync.dma_start(att[:, dc * 512:(dc + 1) * 512], att_sb[:])

    nc.compile()
    return nc


_PROGRAM_CACHE = {}


def _get_program():
    if "nc" not in _PROGRAM_CACHE:
        _PROGRAM_CACHE["nc"] = _build_program()
    return _PROGRAM_CACHE["nc"]


def _block_diag_masks():
    bdm = np.zeros((NQ, 128, BL), dtype=np.float32)
    bdmr = np.zeros((NQ, KRAG, BL), dtype=np.float32)
    for q in range(NQ):
        for bs in range(4):
            bdm[q, bs::4, 4 * q + bs] = 1.0        # rows k = j*4 + bs
            bdmr[q, bs::4, 4 * q + bs] = 1.0
    return bdm, bdmr


def make_in_maps(h, att_feats1, att_feats2, att_masks, W_h, b_h, w_a, b_a):
    h = np.asarray(h, dtype=np.float32)
    att_feats1 = np.asarray(att_feats1, dtype=np.float32)
    att_feats2 = np.asarray(att_feats2, dtype=np.float32)
    att_masks = np.asarray(att_masks, dtype=np.float32)
    W_h = np.ascontiguousarray(np.asarray(W_h, dtype=np.float32))
    b_h = np.asarray(b_h, dtype=np.float32)
    w_a = np.asarray(w_a, dtype=np.float32)
    del b_a  # cancels exactly in the softmax + masked renormalization

    wa4 = np.ascontiguousarray(w_a.reshape(NAC, 128).T)
    bh4 = np.ascontiguousarray(b_h.reshape(NAC, 128).T)
    bdm, bdmr = _block_diag_masks()

    in_maps = []
    for c in range(NCORES):
        sl = slice(c * BL, (c + 1) * BL)
        f1c = att_feats1[sl]
        f2c = att_feats2[sl]
        # f2h[bp, p, i, ac, l] = f2[2bp+i, l, ac*128+p]
        f2h = np.ascontiguousarray(
            f2c.reshape(BL // 2, 2, L, NAC, 128).transpose(0, 4, 1, 3, 2)
        )
        # f1h[q, lc, j, bs, d] = f1[4q+bs, 32lc+j, d]
        f1h = np.ascontiguousarray(
            f1c[:, :LFULL, :].reshape(NQ, 4, NLC, 32, D).transpose(0, 2, 3, 1, 4)
        ).reshape(NQ, NLC, 128, D)
        f1hr = np.ascontiguousarray(
            f1c[:, LFULL:, :].reshape(NQ, 4, LRAG, D).transpose(0, 2, 1, 3)
        ).reshape(NQ, KRAG, D)
        in_maps.append({
            "hT": np.ascontiguousarray(h[sl].T),
            "wh": W_h,
            "bh4": bh4,
            "wa4": wa4,
            "f2h": f2h,
            "f1h": f1h,
            "f1hr": f1hr,
            "msk": np.ascontiguousarray(
                att_masks[sl].reshape(NQ, 4, L).transpose(1, 0, 2)
            ),
            "bdm": bdm,
            "bdmr": bdmr,
        })
    return in_maps


def kernel(h, att_feats1, att_feats2, att_masks, W_h, b_h, w_a, b_a,
           _trace=False, _return_results=False):
    nc = _get_program()
    in_maps = make_in_maps(h, att_feats1, att_feats2, att_masks, W_h, b_h,
                           w_a, b_a)
    res = bass_utils.run_bass_kernel_spmd(
        nc, in_maps, core_ids=list(range(NCORES)), trace=_trace
    )
    out = np.concatenate([res.results[c]["att"] for c in range(NCORES)], axis=0)
    if _return_results:
        return out, res
    return out


# revision 32
# speedup vs baseline: 1.4685x; 1.2266x over previous
"""Trainium2 Bass kernel for nn_Attention_41566693491235.

Computes, for full inputs (B=256, L=196, R=1024, A=512, D=2048):
    att_h  = h @ W_h + b_h                                  [B, A]
    dot    = einsum("bla,a->bl", tanh(f2 + att_h[:,None,:]), w_a) + b_a
    weight = softmax(dot, axis=1) * mask;  weight /= weight.sum(1, keepdims=True)
    att    = einsum("bl,bld->bd", weight, f1)               [B, D]

Sharding: data-parallel over batch, 32 per core x 8 cores. Weights replicated.
Note b_a and the softmax normalizer cancel exactly in the masked renorm:
    weight = exp(dot - max) * mask / sum(exp(dot - max) * mask)

Implementation notes:
  - Matmuls run in float32r (full-rate PE at moving-dim >= 256, ~2e-4 rel err).
  - The weighted sum packs 4 l-rows x all 32 batches per 128-row contraction
    (rows k = b*4 + j) with block-diagonal stationary operands, so all 32
    output rows accumulate partition-aligned in one PSUM tile and L = 49*4
    divides evenly (no ragged chunks).
  - f1 streams on the sync HWDGE ring into a deep SBUF pool from t=0, f2 on
    the scalar ring, so the single softmax barrier never idles the DMA
    engines; small redistribution hops ride gpsimd/scalar.
"""

import numpy as np

import concourse.bass as bass
import concourse.bacc as bacc
import concourse.tile as tile
import concourse.mybir as mybir
from concourse import bass_utils

F32 = mybir.dt.float32
F32R = mybir.dt.float32r
AF = mybir.ActivationFunctionType

# Problem shape (hardcoded; kernel.py must be self-contained).
B, L, R, A, D = 256, 196, 1024, 512, 2048
NCORES = 8
BL = B // NCORES          # 32 batches per core
NLC = L // 4              # 49 l-chunks of 4 rows x 32 batches = 128 K-rows
NAC = A // 128            # 4 chunks of the attention-hidden dim
NKC = R // 128            # 8 chunks of the h-feature dim
NDC = D // 512            # 4 free-dim chunks for the output matmuls


def _build_program(f1_bufs: int = 12, f2_bufs: int = 6):
    nc = bacc.Bacc(
        "TRN2",
        target_bir_lowering=False,
        debug=False,
        enable_asserts=False,
        num_devices=NCORES,
    )

    hT = nc.dram_tensor("hT", [R, BL], F32, kind="ExternalInput").ap()
    wh = nc.dram_tensor("wh", [R, A], F32, kind="ExternalInput").ap()
    bh4 = nc.dram_tensor("bh4", [128, NAC], F32, kind="ExternalInput").ap()
    wa4 = nc.dram_tensor("wa4", [128, NAC], F32, kind="ExternalInput").ap()
    # f2h[bp, p, i, ac, l] = f2[2*bp+i, l, ac*128+p]: one contiguous
    # 800KB block per batch-pair, 6.3KB per partition.
    f2h = nc.dram_tensor("f2h", [BL // 2, 128, 2, NAC, L], F32,
                         kind="ExternalInput").ap()
    # f1h[lch, b, j, d] = f1[b, 4*lch+j, d]: each lch tile is a contiguous
    # 1MB block in exactly the SBUF layout (rows k = b*4 + j).
    f1h = nc.dram_tensor("f1h", [NLC, 128, D], F32, kind="ExternalInput").ap()
    msk = nc.dram_tensor("msk", [BL, L], F32, kind="ExternalInput").ap()
    # bdm[b*4+j, b'] = 1 iff b' == b: the block-diagonal mask.
    bdm = nc.dram_tensor("bdm", [128, BL], F32, kind="ExternalInput").ap()
    att = nc.dram_tensor("att", [BL, D], F32, kind="ExternalOutput").ap()

    with tile.TileContext(nc) as tc:
        with (
            tc.tile_pool(name="const", bufs=1) as cpool,
            tc.tile_pool(name="f2p", bufs=f2_bufs) as f2pool,
            tc.tile_pool(name="ep", bufs=3) as epool,
            tc.tile_pool(name="f1p", bufs=f1_bufs) as f1pool,
            tc.tile_pool(name="small", bufs=1) as spool,
            tc.tile_pool(name="ps", bufs=1, space=bass.MemorySpace.PSUM) as pspool,
            tc.tile_pool(name="psdot", bufs=3, space=bass.MemorySpace.PSUM) as psdot,
            tc.tile_pool(name="dram", bufs=1, space=bass.MemorySpace.DRAM) as dpool,
        ):
            # ---- constants -------------------------------------------------
            wh_t = cpool.tile([128, NKC, A], F32R)
            nc.sync.dma_start(wh_t[:], wh.bitcast(F32R).rearrange("(kc p) a -> p kc a", p=128))
            hT_t = cpool.tile([128, NKC, BL], F32R)
            nc.sync.dma_start(hT_t[:], hT.bitcast(F32R).rearrange("(kc p) b -> p kc b", p=128))
            wa_t = cpool.tile([128, NAC], F32R)
            nc.sync.dma_start(wa_t[:], wa4.bitcast(F32R)[:])
            bh_t = cpool.tile([128, NAC], F32)
            nc.scalar.dma_start(bh_t[:], bh4[:])
            msk_t = cpool.tile([BL, L], F32)
            nc.scalar.dma_start(msk_t[:], msk[:])
            bdm_t = cpool.tile([128, BL], F32)
            nc.scalar.dma_start(bdm_t[:], bdm[:])

            # DRAM scratch for the partition-redistribution round-trips.
            dot_dram = dpool.tile([BL, L], F32)
            w_dram = dpool.tile([NLC, BL, 4], F32)

            # ---- phase 1: att_h.T = W_h.T @ h.T (+ b_h) --------------------
            # atth[:, ac, b] holds att_h[b, ac*128 + p] on partition p.
            ps_atth = pspool.tile([128, NAC, BL], F32)
            for mc in range(NAC):
                for kc in range(NKC):
                    nc.tensor.matmul(
                        ps_atth[:, mc, :],
                        wh_t[:, kc, mc * 128:(mc + 1) * 128],
                        hT_t[:, kc, :],
                        start=(kc == 0),
                        stop=(kc == NKC - 1),
                    )
            atth = cpool.tile([128, NAC, BL], F32)
            for mc in range(NAC):
                nc.vector.tensor_scalar_add(
                    atth[:, mc, :], ps_atth[:, mc, :], bh_t[:, mc:mc + 1]
                )

            # ---- phase 2: tanh + dot, two batches per matmul ---------------
            # dot[b, l] = sum_a tanh(f2[b,l,a] + att_h[b,a]) * w_a[a]
            for bp in range(BL // 2):
                f2b = f2pool.tile([128, 2, NAC, L], F32, tag="f2b",
                                  name=f"f2b{bp}")
                nc.scalar.dma_start(f2b[:], f2h[bp])
                e2 = epool.tile([128, NAC, 2, L], F32R, tag="e2",
                                name=f"e2{bp}")
                for i in range(2):
                    for ac in range(NAC):
                        nc.scalar.activation(
                            e2[:, ac, i, :], f2b[:, i, ac, :], AF.Tanh,
                            bias=atth[:, ac, 2 * bp + i:2 * bp + i + 1],
                        )
                pd = psdot.tile([1, 2, L], F32, tag="pd", name=f"pd{bp}")
                for ac in range(NAC):
                    nc.tensor.matmul(
                        pd[:], wa_t[:, ac:ac + 1], e2[:, ac, :, :],
                        start=(ac == 0), stop=(ac == NAC - 1),
                    )
                dotflat = spool.tile([1, 2 * L], F32, tag="dotflat", bufs=3,
                                     name=f"dotflat{bp}")
                nc.vector.tensor_copy(dotflat[:], pd[:])
                nc.gpsimd.dma_start(dot_dram[2 * bp:2 * bp + 2, :], dotflat[:])

            # ---- phase 3: batched masked softmax (all 32 batches) ----------
            dott = spool.tile([BL, L], F32)
            nc.scalar.dma_start(dott[:], dot_dram[:])
            negmax = spool.tile([BL, 1], F32)
            nc.vector.tensor_reduce(
                negmax[:], dott[:], axis=mybir.AxisListType.X,
                op=mybir.AluOpType.max, negate=True,
            )
            wexp = spool.tile([BL, L], F32)
            nc.scalar.activation(wexp[:], dott[:], AF.Exp, bias=negmax[:])
            wm = spool.tile([BL, L], F32)
            nc.vector.tensor_mul(wm[:], wexp[:], msk_t[:])
            ssum = spool.tile([BL, 1], F32)
            nc.vector.reduce_sum(ssum[:], wm[:], axis=mybir.AxisListType.X)
            sinv = spool.tile([BL, 1], F32)
            nc.vector.reciprocal(sinv[:], ssum[:])
            wn = spool.tile([BL, L], F32)
            nc.vector.tensor_scalar_mul(wn[:], wm[:], sinv[:])

            # ---- phase 4: redistribute weights into block-diag layout ------
            # W2[b*4+j, lch] = wn[b, 4*lch+j], staged through DRAM.
            nc.scalar.dma_start(
                w_dram.rearrange("lch b j -> b lch j"), wn[:]
            )
            w2 = spool.tile([128, NLC], F32)
            nc.scalar.dma_start(w2[:], w_dram.rearrange("lch b j -> (b j) lch"))
            ldt = spool.tile([128, NLC, BL], F32R)
            for lch in range(NLC):
                nc.vector.tensor_scalar_mul(
                    ldt[:, lch, :], bdm_t[:], w2[:, lch:lch + 1]
                )

            # ---- phase 5: att = weight @ f1, all 32 batches per matmul -----
            ps_att = pspool.tile([BL, NDC, 512], F32)
            for lch in range(NLC):
                f1t = f1pool.tile([128, D], F32R, tag="f1t", name=f"f1t{lch}")
                nc.sync.dma_start(f1t[:], f1h.bitcast(F32R)[lch])
                for dc in range(NDC):
                    nc.tensor.matmul(
                        ps_att[:, dc, :],
                        ldt[:, lch, :],
                        f1t[:, dc * 512:(dc + 1) * 512],
                        start=(lch == 0),
                        stop=(lch == NLC - 1),
                    )

            for dc in range(NDC):
                att_sb = spool.tile([BL, 512], F32, tag="att_sb", bufs=2,
                                    name=f"att_sb{dc}")
                nc.vector.tensor_copy(att_sb[:], ps_att[:, dc, :])
                nc.sync.dma_start(att[:, dc * 512:(dc + 1) * 512], att_sb[:])

    nc.compile()
    return nc


_PROGRAM_CACHE = {}


def _get_program():
    if "nc" not in _PROGRAM_CACHE:
        _PROGRAM_CACHE["nc"] = _build_program()
    return _PROGRAM_CACHE["nc"]


def make_in_maps(h, att_feats1, att_feats2, att_masks, W_h, b_h, w_a, b_a):
    h = np.asarray(h, dtype=np.float32)
    att_feats1 = np.asarray(att_feats1, dtype=np.float32)
    att_feats2 = np.asarray(att_feats2, dtype=np.float32)
    att_masks = np.asarray(att_masks, dtype=np.float32)
    W_h = np.ascontiguousarray(np.asarray(W_h, dtype=np.float32))
    b_h = np.asarray(b_h, dtype=np.float32)
    w_a = np.asarray(w_a, dtype=np.float32)
    del b_a  # cancels exactly in the softmax + masked renormalization

    wa4 = np.ascontiguousarray(w_a.reshape(NAC, 128).T)
    bh4 = np.ascontiguousarray(b_h.reshape(NAC, 128).T)
    bdm = np.zeros((128, BL), dtype=np.float32)
    for b in range(BL):
        bdm[b * 4:(b + 1) * 4, b] = 1.0

    in_maps = []
    for c in range(NCORES):
        sl = slice(c * BL, (c + 1) * BL)
        f1c = att_feats1[sl]
        f2c = att_feats2[sl]
        # f2h[bp, p, i, ac, l] = f2[2bp+i, l, ac*128+p]
        f2h = np.ascontiguousarray(
            f2c.reshape(BL // 2, 2, L, NAC, 128).transpose(0, 4, 1, 3, 2)
        )
        # f1h[lch, b, j, d] = f1[b, 4lch+j, d]
        f1h = np.ascontiguousarray(
            f1c.reshape(BL, NLC, 4, D).transpose(1, 0, 2, 3)
        ).reshape(NLC, 128, D)
        in_maps.append({
            "hT": np.ascontiguousarray(h[sl].T),
            "wh": W_h,
            "bh4": bh4,
            "wa4": wa4,
            "f2h": f2h,
            "f1h": f1h,
            "msk": np.ascontiguousarray(att_masks[sl]),
            "bdm": bdm,
        })
    return in_maps


def kernel(h, att_feats1, att_feats2, att_masks, W_h, b_h, w_a, b_a,
           _trace=False, _return_results=False):
    nc = _get_program()
    in_maps = make_in_maps(h, att_feats1, att_feats2, att_masks, W_h, b_h,
                           w_a, b_a)
    res = bass_utils.run_bass_kernel_spmd(
        nc, in_maps, core_ids=list(range(NCORES)), trace=_trace
    )
    out = np.concatenate([res.results[c]["att"] for c in range(NCORES)], axis=0)
    if _return_results:
        return out, res
    return out


# revision 33
# speedup vs baseline: 1.6229x; 1.1051x over previous
"""Trainium2 Bass kernel for nn_Attention_41566693491235.

Computes, for full inputs (B=256, L=196, R=1024, A=512, D=2048):
    att_h  = h @ W_h + b_h                                  [B, A]
    dot    = einsum("bla,a->bl", tanh(f2 + att_h[:,None,:]), w_a) + b_a
    weight = softmax(dot, axis=1) * mask;  weight /= weight.sum(1, keepdims=True)
    att    = einsum("bl,bld->bd", weight, f1)               [B, D]

Sharding: data-parallel over batch, 32 per core x 8 cores. Weights replicated.
Note b_a and the softmax normalizer cancel exactly in the masked renorm:
    weight = exp(dot - max) * mask / sum(exp(dot - max) * mask)

Implementation notes:
  - Matmuls run in float32r (full-rate PE at moving-dim >= 256, ~2e-4 rel err).
  - The weighted sum packs 4 l-rows x all 32 batches per 128-row contraction
    (rows k = b*4 + j) with block-diagonal stationary operands, so all 32
    output rows accumulate partition-aligned in one PSUM tile and L = 49*4
    divides evenly (no ragged chunks).
  - f1 streams on the sync HWDGE ring into a deep SBUF pool from t=0; f2
    alternates between the scalar HWDGE ring and gpsimd SWDGE; softmax runs
    per 16-batch half as soon as that half's dots land, so only the last
    half's weight redistribution sits on the critical path.
"""

import numpy as np

import concourse.bass as bass
import concourse.bacc as bacc
import concourse.tile as tile
import concourse.mybir as mybir
from concourse import bass_utils

F32 = mybir.dt.float32
F32R = mybir.dt.float32r
AF = mybir.ActivationFunctionType

# Problem shape (hardcoded; kernel.py must be self-contained).
B, L, R, A, D = 256, 196, 1024, 512, 2048
NCORES = 8
BL = B // NCORES          # 32 batches per core
NLC = L // 4              # 49 l-chunks of 4 rows x 32 batches = 128 K-rows
NAC = A // 128            # 4 chunks of the attention-hidden dim
NKC = R // 128            # 8 chunks of the h-feature dim
NDC = D // 512            # 4 free-dim chunks for the output matmuls


def _build_program(f1_bufs: int = 16, f2_bufs: int = 4):
    nc = bacc.Bacc(
        "TRN2",
        target_bir_lowering=False,
        debug=False,
        enable_asserts=False,
        num_devices=NCORES,
    )

    hT = nc.dram_tensor("hT", [R, BL], F32, kind="ExternalInput").ap()
    wh = nc.dram_tensor("wh", [R, A], F32, kind="ExternalInput").ap()
    bh4 = nc.dram_tensor("bh4", [128, NAC], F32, kind="ExternalInput").ap()
    wa4 = nc.dram_tensor("wa4", [128, NAC], F32, kind="ExternalInput").ap()
    # f2h[bp, p, i, ac, l] = f2[2*bp+i, l, ac*128+p]: one contiguous
    # 800KB block per batch-pair, 6.3KB per partition.
    f2h = nc.dram_tensor("f2h", [BL // 2, 128, 2, NAC, L], F32,
                         kind="ExternalInput").ap()
    # f1h[lch, b, j, d] = f1[b, 4*lch+j, d]: each lch tile is a contiguous
    # 1MB block in exactly the SBUF layout (rows k = b*4 + j).
    f1h = nc.dram_tensor("f1h", [NLC, 128, D], F32, kind="ExternalInput").ap()
    # msk2[bs, half, l] = att_masks[16*half + bs, l]
    msk = nc.dram_tensor("msk", [16, 2, L], F32, kind="ExternalInput").ap()
    # bdm[b*4+j, b'] = 1 iff b' == b: the block-diagonal mask.
    bdm = nc.dram_tensor("bdm", [128, BL], F32, kind="ExternalInput").ap()
    att = nc.dram_tensor("att", [BL, D], F32, kind="ExternalOutput").ap()

    with tile.TileContext(nc) as tc:
        with (
            tc.tile_pool(name="const", bufs=1) as cpool,
            tc.tile_pool(name="f2p", bufs=f2_bufs) as f2pool,
            tc.tile_pool(name="ep", bufs=3) as epool,
            tc.tile_pool(name="f1p", bufs=f1_bufs) as f1pool,
            tc.tile_pool(name="small", bufs=1) as spool,
            tc.tile_pool(name="ps", bufs=1, space=bass.MemorySpace.PSUM) as pspool,
            tc.tile_pool(name="psdot", bufs=3, space=bass.MemorySpace.PSUM) as psdot,
            tc.tile_pool(name="dram", bufs=1, space=bass.MemorySpace.DRAM) as dpool,
        ):
            # ---- constants -------------------------------------------------
            # W_h rides in f1-pool slots (released after phase 1, so the
            # space is recycled for f1 buffering).
            wh_tiles = []
            for kc in range(NKC):
                wht = f1pool.tile([128, A], F32R, tag="f1t", name=f"wh{kc}")
                nc.sync.dma_start(
                    wht[:], wh.bitcast(F32R)[kc * 128:(kc + 1) * 128, :]
                )
                wh_tiles.append(wht)
            hT_t = cpool.tile([128, NKC, BL], F32R)
            nc.sync.dma_start(hT_t[:], hT.bitcast(F32R).rearrange("(kc p) b -> p kc b", p=128))
            wa_t = cpool.tile([128, NAC], F32R)
            nc.sync.dma_start(wa_t[:], wa4.bitcast(F32R)[:])
            bh_t = cpool.tile([128, NAC], F32)
            nc.scalar.dma_start(bh_t[:], bh4[:])
            msk_t = cpool.tile([16, 2, L], F32)
            nc.scalar.dma_start(msk_t[:], msk[:])
            bdm_t = cpool.tile([128, BL], F32)
            nc.scalar.dma_start(bdm_t[:], bdm[:])

            # DRAM scratch for the partition-redistribution round-trips.
            dot_dram = dpool.tile([BL, L], F32)
            w_dram = dpool.tile([NLC, BL, 4], F32)

            # ---- phase 1: att_h.T = W_h.T @ h.T (+ b_h) --------------------
            # atth[:, ac, b] holds att_h[b, ac*128 + p] on partition p.
            ps_atth = pspool.tile([128, NAC, BL], F32)
            for mc in range(NAC):
                for kc in range(NKC):
                    nc.tensor.matmul(
                        ps_atth[:, mc, :],
                        wh_tiles[kc][:, mc * 128:(mc + 1) * 128],
                        hT_t[:, kc, :],
                        start=(kc == 0),
                        stop=(kc == NKC - 1),
                    )
            atth = cpool.tile([128, NAC, BL], F32)
            for mc in range(NAC):
                nc.vector.tensor_scalar_add(
                    atth[:, mc, :], ps_atth[:, mc, :], bh_t[:, mc:mc + 1]
                )

            # ---- phase 2: tanh + dot, two batches per matmul ---------------
            # dot[b, l] = sum_a tanh(f2[b,l,a] + att_h[b,a]) * w_a[a]
            def emit_pair(bp):
                f2b = f2pool.tile([128, 2, NAC, L], F32, tag="f2b",
                                  name=f"f2b{bp}")
                dma_eng = nc.scalar if bp % 2 == 0 else nc.gpsimd
                dma_eng.dma_start(f2b[:], f2h[bp])
                e2 = epool.tile([128, NAC, 2, L], F32R, tag="e2",
                                name=f"e2{bp}")
                for i in range(2):
                    for ac in range(NAC):
                        nc.scalar.activation(
                            e2[:, ac, i, :], f2b[:, i, ac, :], AF.Tanh,
                            bias=atth[:, ac, 2 * bp + i:2 * bp + i + 1],
                        )
                pd = psdot.tile([1, 2, L], F32, tag="pd", name=f"pd{bp}")
                for ac in range(NAC):
                    nc.tensor.matmul(
                        pd[:], wa_t[:, ac:ac + 1], e2[:, ac, :, :],
                        start=(ac == 0), stop=(ac == NAC - 1),
                    )
                dotflat = spool.tile([1, 2 * L], F32, tag="dotflat", bufs=3,
                                     name=f"dotflat{bp}")
                nc.vector.tensor_copy(dotflat[:], pd[:])
                nc.gpsimd.dma_start(dot_dram[2 * bp:2 * bp + 2, :], dotflat[:])

            # ---- phase 3/4 per 16-batch half: masked softmax + weight
            # redistribution write. Only the last half's chain gates step5.
            def emit_half(hf):
                b0 = 16 * hf
                dott = spool.tile([16, L], F32, tag=f"dott{hf}",
                                  name=f"dott{hf}")
                nc.scalar.dma_start(dott[:], dot_dram[b0:b0 + 16, :])
                negmax = spool.tile([16, 1], F32, tag=f"negmax{hf}",
                                    name=f"negmax{hf}")
                nc.vector.tensor_reduce(
                    negmax[:], dott[:], axis=mybir.AxisListType.X,
                    op=mybir.AluOpType.max, negate=True,
                )
                wexp = spool.tile([16, L], F32, tag=f"wexp{hf}",
                                  name=f"wexp{hf}")
                nc.scalar.activation(wexp[:], dott[:], AF.Exp, bias=negmax[:])
                wm = spool.tile([16, L], F32, tag=f"wm{hf}", name=f"wm{hf}")
                nc.vector.tensor_mul(wm[:], wexp[:], msk_t[:, hf, :])
                ssum = spool.tile([16, 1], F32, tag=f"ssum{hf}",
                                  name=f"ssum{hf}")
                nc.vector.reduce_sum(ssum[:], wm[:], axis=mybir.AxisListType.X)
                sinv = spool.tile([16, 1], F32, tag=f"sinv{hf}",
                                  name=f"sinv{hf}")
                nc.vector.reciprocal(sinv[:], ssum[:])
                wn = spool.tile([16, L], F32, tag=f"wn{hf}", name=f"wn{hf}")
                nc.vector.tensor_scalar_mul(wn[:], wm[:], sinv[:])
                # W2[b*4+j, lch] = wn[b, 4*lch+j], staged through DRAM.
                nc.scalar.dma_start(
                    w_dram[:, b0:b0 + 16, :].rearrange("lch b j -> b lch j"),
                    wn[:],
                )

            for bp in range(8):
                emit_pair(bp)
            emit_half(0)
            for bp in range(8, 16):
                emit_pair(bp)
            emit_half(1)

            w2 = spool.tile([128, NLC], F32)
            nc.scalar.dma_start(w2[:], w_dram.rearrange("lch b j -> (b j) lch"))
            ldt = spool.tile([128, NLC, BL], F32R)
            for lch in range(NLC):
                nc.vector.tensor_scalar_mul(
                    ldt[:, lch, :], bdm_t[:], w2[:, lch:lch + 1]
                )

            # ---- phase 5: att = weight @ f1, all 32 batches per matmul -----
            ps_att = pspool.tile([BL, NDC, 512], F32)
            for lch in range(NLC):
                f1t = f1pool.tile([128, D], F32R, tag="f1t", name=f"f1t{lch}")
                nc.sync.dma_start(f1t[:], f1h.bitcast(F32R)[lch])
                for dc in range(NDC):
                    nc.tensor.matmul(
                        ps_att[:, dc, :],
                        ldt[:, lch, :],
                        f1t[:, dc * 512:(dc + 1) * 512],
                        start=(lch == 0),
                        stop=(lch == NLC - 1),
                    )

            for dc in range(NDC):
                att_sb = spool.tile([BL, 512], F32, tag="att_sb", bufs=2,
                                    name=f"att_sb{dc}")
                nc.vector.tensor_copy(att_sb[:], ps_att[:, dc, :])
                nc.sync.dma_start(att[:, dc * 512:(dc + 1) * 512], att_sb[:])

    nc.compile()
    return nc


_PROGRAM_CACHE = {}


def _get_program():
    if "nc" not in _PROGRAM_CACHE:
        _PROGRAM_CACHE["nc"] = _build_program()
    return _PROGRAM_CACHE["nc"]


def make_in_maps(h, att_feats1, att_feats2, att_masks, W_h, b_h, w_a, b_a):
    h = np.asarray(h, dtype=np.float32)
    att_feats1 = np.asarray(att_feats1, dtype=np.float32)
    att_feats2 = np.asarray(att_feats2, dtype=np.float32)
    att_masks = np.asarray(att_masks, dtype=np.float32)
    W_h = np.ascontiguousarray(np.asarray(W_h, dtype=np.float32))
    b_h = np.asarray(b_h, dtype=np.float32)
    w_a = np.asarray(w_a, dtype=np.float32)
    del b_a  # cancels exactly in the softmax + masked renormalization

    wa4 = np.ascontiguousarray(w_a.reshape(NAC, 128).T)
    bh4 = np.ascontiguousarray(b_h.reshape(NAC, 128).T)
    bdm = np.zeros((128, BL), dtype=np.float32)
    for b in range(BL):
        bdm[b * 4:(b + 1) * 4, b] = 1.0

    in_maps = []
    for c in range(NCORES):
        sl = slice(c * BL, (c + 1) * BL)
        f1c = att_feats1[sl]
        f2c = att_feats2[sl]
        # f2h[bp, p, i, ac, l] = f2[2bp+i, l, ac*128+p]
        f2h = np.ascontiguousarray(
            f2c.reshape(BL // 2, 2, L, NAC, 128).transpose(0, 4, 1, 3, 2)
        )
        # f1h[lch, b, j, d] = f1[b, 4lch+j, d]
        f1h = np.ascontiguousarray(
            f1c.reshape(BL, NLC, 4, D).transpose(1, 0, 2, 3)
        ).reshape(NLC, 128, D)
        in_maps.append({
            "hT": np.ascontiguousarray(h[sl].T),
            "wh": W_h,
            "bh4": bh4,
            "wa4": wa4,
            "f2h": f2h,
            "f1h": f1h,
            "msk": np.ascontiguousarray(
                att_masks[sl].reshape(2, 16, L).transpose(1, 0, 2)
            ),
            "bdm": bdm,
        })
    return in_maps


def kernel(h, att_feats1, att_feats2, att_masks, W_h, b_h, w_a, b_a,
           _trace=False, _return_results=False):
    nc = _get_program()
    in_maps = make_in_maps(h, att_feats1, att_feats2, att_masks, W_h, b_h,
                           w_a, b_a)
    res = bass_utils.run_bass_kernel_spmd(
        nc, in_maps, core_ids=list(range(NCORES)), trace=_trace
    )
    out = np.concatenate([res.results[c]["att"] for c in range(NCORES)], axis=0)
    if _return_results:
        return out, res
    return out
